# revision 31
# baseline (speedup 1.0000x reference)
"""Trainium2 Bass kernel for nn_Model_22677427323544.

The circuit is AngleEmbedding(adds) followed by a batch-independent gate
sequence, then <Z_0>. Algebraically out[b] = r_b^T A r_b with A a fixed real
symmetric 512x512 matrix and r_b the real Kronecker vector of per-wire
(cos(t/2), sin(t/2)).

Each wire contributes a factor c^2, s^2, or c*s to every A[j,k] r_j r_k term,
so the quadratic form collapses to a LINEAR form over per-wire 3-vectors.
Since (c^2, s^2, c*s) = T (1, cos t, sin t) with a fixed 3x3 T, fold T into
the coefficient tensor on the host: with h_i[b] = (1, cos t_i, sin t_i),

    out[b] = < A3h , h_0[b] x h_1[b] x ... x h_8[b] >

Split wires 0-3 (81) / 4-8 (243):  out[b] = H_hi[b]^T A3h H_lo[b].
The h-basis needs NO device-side work beyond two Sin activations (the ones
plane is a memset), unlike the g-basis which needs squares/products.

Device (per core, 1024 samples = 128 partitions x 8 groups; wire slots
host-permuted to [0,5,2,7,1,6,3,8,4] so every kron level reads contiguous
slices):
  1. half-angle trig keeps Sin-table inputs in the proven range:
     w=sin(t/2), u=sin(t/4); cos t = 1-2w^2 (GpSimd), sin t = 2w(1-2u^2)
     (DVE).  A dummy activation hoists the 1.3us Sin ACT_TABLE_LOAD off the
     adds-DMA critical path.  hv[P,G,3,9] fp32, ones plane via memset.
  2. DVE krons (fp32 -- fp16 gives no DVE speedup on broadcast APs and
     ~20% penalty): q[P,G,4,9] wire-pairs (3 ops, ISA caps free dims at 3),
     ghi[P,G,81] (fp16 out, for the PE) + lo4[P,G,81] (fp32), glo =
     h4 x lo4 (groups 0-3 in one DVE op; 4-7 per-group on GpSimd, which
     runs ~570ns/group steady-state after a warm-up op absorbs the ~1.2us
     Q7 library-load penalty).
  3. all 8 PE transposes of ghi_g first (fp16 -> 1 cyc/row; emitting any
     copy-gated matmul earlier stalls the in-order PE) with per-2-group
     ScalarE PSUM->SBUF casts, then 8 fp16 matmuls Y_g = ghi_g^T @ A3h,
     one PSUM tile per group (a shared tile would make every STT wait on
     all 8 matmuls).
  4. per group: fused VectorE scalar_tensor_tensor dot rowsum(Y_g * glo_g);
     each half of res is DMAed out as soon as its STTs finish.
DMAs: adds dispatched on SP, A3h on ScalarE (parallel dispatch); out on SP.
The exec-time window is (last event incl ~9.9us of fixed out-DMA latency +
framework teardown) - (first engine instruction); the only controllable
term is input-DMA wait + the VectorE serial chain, which now runs gap-free.
"""
import math

import numpy as np

import concourse.bass as bass
import concourse.tile as tile
from concourse import bacc, mybir
from concourse import bass_utils

N_WIRES = 9
N_CORES = 8
B = 8192
B_LOC = B // N_CORES          # 1024
P = 128                       # partitions
G = B_LOC // P                # 8 batch groups per partition
NH = 81                       # 3^4, wires 0-3
NL = 243                      # 3^5, wires 4-8
NLP = 256                     # padded so each PSUM matmul slab is half a bank
F32 = mybir.dt.float32
F16 = mybir.dt.float16

# wire -> hv slot order: slots 0-3 = hi factors of pairs (0,1),(5,6),(2,3),
# (7,8); slots 4-7 = lo factors; slot 8 = wire 4.  This makes every kron
# level read contiguous slot slices (see _build_program).
PERM = [0, 5, 2, 7, 1, 6, 3, 8, 4]

# ---------------------------------------------------------------------------
# Host-side parameter folding: A = Re(D^H U^H Z0 U D), 3-ary fold, T fold
# ---------------------------------------------------------------------------

DIM = 1 << N_WIRES

_X = np.array([[0, 1], [1, 0]], dtype=np.complex128)
_CNOT = np.array(
    [[1, 0, 0, 0], [0, 1, 0, 0], [0, 0, 0, 1], [0, 0, 1, 0]], dtype=np.complex128
)


def _rx(t):
    c, s = np.cos(t / 2), np.sin(t / 2)
    return np.array([[c, -1j * s], [-1j * s, c]])


def _ry(t):
    c, s = np.cos(t / 2), np.sin(t / 2)
    return np.array([[c, -s], [s, c]], dtype=np.complex128)


def _rz(t):
    return np.array([[np.exp(-0.5j * t), 0], [0, np.exp(0.5j * t)]])


def _rot(phi, theta, omega):
    return _rz(omega) @ _ry(theta) @ _rz(phi)


def _crz(t):
    return np.diag([1, 1, np.exp(-0.5j * t), np.exp(0.5j * t)]).astype(np.complex128)


def _crx(t):
    m = np.eye(4, dtype=np.complex128)
    m[2:, 2:] = _rx(t)
    return m


def _apply_1q(state, U, w):
    s = np.moveaxis(state, 1 + w, -1)
    s = np.einsum('ij,...j->...i', U, s)
    return np.moveaxis(s, -1, 1 + w)


def _apply_2q(state, U, c, t):
    s = np.moveaxis(state, (1 + c, 1 + t), (-2, -1))
    shp = s.shape
    s = s.reshape(shp[:-2] + (4,))
    s = np.einsum('ij,...j->...i', U, s)
    return np.moveaxis(s.reshape(shp), (-2, -1), (1 + c, 1 + t))


def _entangle_block(state, p):
    j = 0
    for i in range(N_WIRES):
        ip = (i + 1) % N_WIRES
        state = _apply_1q(state, _ry(p[j]), i)
        state = _apply_1q(state, _ry(p[j + 1]), ip)
        state = _apply_2q(state, _CNOT, i, ip)
        state = _apply_2q(state, _crz(p[j + 2]), i, ip)
        state = _apply_1q(state, _X, ip)
        state = _apply_2q(state, _crx(p[j + 3]), i, ip)
        j += 4
    return state


def _sel_layer(state, w, r):
    for i in range(N_WIRES):
        state = _apply_1q(state, _rot(w[i, 0], w[i, 1], w[i, 2]), i)
    for i in range(N_WIRES):
        state = _apply_2q(state, _CNOT, i, (i + r) % N_WIRES)
    return state


def _compute_A(params, weights, params2):
    """Return the folded h-basis coefficient matrix A3h [81, 256] (fp16)."""
    params = np.asarray(params, np.float64)
    weights = np.asarray(weights, np.float64)
    params2 = np.asarray(params2, np.float64)
    state = np.eye(DIM, dtype=np.complex128).reshape((DIM,) + (2,) * N_WIRES)
    for l in range(3):
        state = _entangle_block(state, params[l * 36:(l + 1) * 36])
    for l in range(3):
        state = _sel_layer(state, weights[l], (l % (N_WIRES - 1)) + 1)
    for l in range(5):
        state = _entangle_block(state, params2[l * 36:(l + 1) * 36])
    U = state.reshape(DIM, DIM).T
    z = np.where(np.arange(DIM) < DIM // 2, 1.0, -1.0)
    M = U.conj().T @ (z[:, None] * U)
    pc = np.array([bin(j).count('1') for j in range(DIM)])
    d = (-1j) ** pc
    A = ((np.conj(d)[:, None] * M * d[None, :]).real).astype(np.float64)

    # fold 512x512 -> 3^9: digit 0 = (0,0), 1 = (1,1), 2 = (0,1)/(1,0)
    j = np.arange(DIM)
    jb = (j[:, None, None] >> (8 - np.arange(N_WIRES))[None, None, :]) & 1
    kb = (j[None, :, None] >> (8 - np.arange(N_WIRES))[None, None, :]) & 1
    digit = np.where((jb == 0) & (kb == 0), 0, np.where((jb == 1) & (kb == 1), 1, 2))
    m = np.zeros((DIM, DIM), np.int64)
    for i in range(N_WIRES):
        m = m * 3 + digit[:, :, i]
    A3 = np.zeros(3 ** N_WIRES)
    np.add.at(A3, m.ravel(), A.ravel())

    # change of basis per wire: g = (c^2, s^2, cs) = T (1, cos t, sin t)
    T = np.array([[.5, .5, 0.], [.5, -.5, 0.], [0., 0., .5]])
    A9 = A3.reshape((3,) * N_WIRES)
    for ax in range(N_WIRES):
        A9 = np.moveaxis(np.tensordot(A9, T, axes=([ax], [0])), -1, ax)
    A3h = A9.reshape(NH, NL)
    A3p = np.zeros((NH, NLP), np.float16)
    A3p[:, :NL] = A3h.astype(np.float16)
    return np.ascontiguousarray(A3p)


# ---------------------------------------------------------------------------
# Device program (per core: 1024 samples; sample index = p*G + g)
# ---------------------------------------------------------------------------

_PROGRAM = None


def _build_program():
    nc = bacc.Bacc("TRN2", target_bir_lowering=False, debug=False,
                   num_devices=N_CORES)
    adds_ext = nc.dram_tensor("adds", [B_LOC, N_WIRES], F32,
                              kind="ExternalInput").ap()
    amat_ext = nc.dram_tensor("amat", [NH, NLP], F16,
                              kind="ExternalInput").ap()
    out_ext = nc.dram_tensor("out", [B_LOC], F32, kind="ExternalOutput").ap()

    SIN = mybir.ActivationFunctionType.Sin

    with tile.TileContext(nc) as tc:
        with (
            tc.tile_pool(name="const", bufs=1) as cpool,
            tc.tile_pool(name="psum_t", bufs=2, space="PSUM") as pt,
            tc.tile_pool(name="psum_y", bufs=4, space="PSUM") as py,
        ):
            # input DMAs: adds on SP, A3h on Scalar -- parallel dispatch
            adds_sb = cpool.tile([P, G, N_WIRES], F32)
            nc.sync.dma_start(adds_sb[:], adds_ext.rearrange("(p g) i -> p g i", g=G))

            # identity for PE transpose (fp16 to match transposed data)
            ident = cpool.tile([P, P], F16)
            nc.gpsimd.memset(ident[:], 0.0)
            nc.gpsimd.affine_select(
                out=ident[:], in_=ident[:],
                compare_op=mybir.AluOpType.not_equal, fill=1.0,
                base=0, pattern=[[-1, P]], channel_multiplier=1)

            # hv[p,g,comp,slot]: comp 0 = 1, 1 = cos t, 2 = sin t
            hv = cpool.tile([P, G, 3, N_WIRES], F32)
            nc.gpsimd.memset(hv[:, :, 0, :], 1.0)

            # GpSimd warm-up: the first Pool tensor op pays a ~1.2us Q7
            # library-load penalty, and the first op of the glo shape pays
            # another ~0.9us first-use cost -- pay both here, during the DMA
            # wait, so the real glo ops run at steady-state ~570ns
            pwarm = cpool.tile([P, 2], F32)
            nc.gpsimd.tensor_mul(pwarm[:], hv[:, 0, 0, 0:2], hv[:, 0, 0, 0:2])
            pwarm2 = cpool.tile([P, 3, NH], F32)
            nc.gpsimd.tensor_mul(
                pwarm2[:],
                hv[:, 0, 0, 0][:, None, None].to_broadcast((P, 3, NH)),
                hv[:, 0, 0, 1][:, None, None].to_broadcast((P, 3, NH)))

            # dummy activation on a ready tile: the act-table insert pass
            # attaches the following activation's waits to the table load, so
            # give it one with no data deps and the 1.3us Sin ACT_TABLE_LOAD
            # runs during the adds-DMA wait
            sdum = cpool.tile([P, 1], F32)
            nc.scalar.activation(sdum[:], ident[:, 0:1], SIN, scale=1.0)
            # a3 on SP after adds: keeps ScalarE free for the trig chain
            a3_sb = cpool.tile([NH, NLP], F16)
            nc.sync.dma_start(a3_sb[:], amat_ext)

            # half-angle trig (Sin inputs stay within the proven table range):
            # w = sin(t/2), u = sin(t/4); cos t = 1-2w^2,
            # sin t = 2w(1-2u^2).  u is computed FIRST so DVE's sin-t path
            # (usq, c2) overlaps the w activation; the cos-t path runs
            # entirely on ScalarE (Square, then Identity with scale/bias --
            # both live in the already-loaded trig table set).
            w = cpool.tile([P, G, N_WIRES], F32)
            u = cpool.tile([P, G, N_WIRES], F32)
            nc.scalar.activation(u[:], adds_sb[:], SIN, scale=0.25)
            nc.scalar.activation(w[:], adds_sb[:], SIN, scale=0.5)
            wsq = cpool.tile([P, G, N_WIRES], F32)
            usq = cpool.tile([P, G, N_WIRES], F32)
            c2 = cpool.tile([P, G, N_WIRES], F32)
            nc.scalar.square(wsq[:], w[:])
            nc.scalar.activation(hv[:, :, 1, :], wsq[:],
                                 mybir.ActivationFunctionType.Identity,
                                 scale=-2.0, bias=1.0)
            nc.vector.tensor_mul(usq[:], u[:], u[:])
            nc.vector.tensor_scalar(
                out=c2[:], in0=usq[:], scalar1=-2.0, scalar2=1.0,
                op0=mybir.AluOpType.mult, op1=mybir.AluOpType.add)
            nc.vector.scalar_tensor_tensor(
                out=hv[:, :, 2, :], in0=w[:], scalar=2.0, in1=c2[:],
                op0=mybir.AluOpType.mult, op1=mybir.AluOpType.mult)

            # q[p,g,j,3b+m] = hv[p,g,b,j] * hv[p,g,m,4+j], one DVE op per b
            # (DVE ISA caps free dims at 3).  j order: pairs (w0,w1),(w5,w6),
            # (w2,w3),(w7,w8).
            q = cpool.tile([P, G, 4, 9], F32)
            q_lo = hv[:, :, :, 4:8].rearrange("p g m j -> p g j m")
            q_b = q[:].rearrange("p g j (b m) -> p g j b m", b=3)
            # the b=0 plane is a multiply by ones, i.e. a copy: ScalarE
            # (idle here) does it while DVE does the two real products
            nc.scalar.copy(q_b[:, :, :, 0, :], q_lo)
            for b in (1, 2):
                q_hi = hv[:, :, b, 0:4][:, :, :, None].to_broadcast((P, G, 4, 3))
                nc.vector.tensor_mul(q_b[:, :, :, b, :], q_hi, q_lo)

            # ghi[p,g,9B+M] = q0[B]*q2[M] (digits d0d1 d2d3) -- fp16 so the
            # PE transposes run at 1 cyc/row instead of 2
            # lo4[p,g,9B+M] = q1[B]*q3[M] (digits d5d6 d7d8) -- fp32 for glo
            ghi = cpool.tile([P, G, NH], F16)
            lo4 = cpool.tile([P, G, NH], F32)
            for k, dst in ((0, ghi), (1, lo4)):
                rr_out = dst[:].rearrange("p g (B M) -> p g B M", B=9)
                rr_hi = q[:, :, k, :][:, :, :, None].to_broadcast((P, G, 9, 9))
                rr_lo = q[:, :, 2 + k, :][:, :, None, :].to_broadcast((P, G, 9, 9))
                nc.vector.tensor_mul(rr_out, rr_hi, rr_lo)

            # per 2 groups: PE transpose ghi_g (fp16, 1 cyc/row), cast to
            # fp16 SBUF on ScalarE.  All 8 transposes are emitted before any
            # matmul: they are cheap, and the in-order PE otherwise stalls on
            # a copy-gated matmul while later transposes (and hence later
            # copies and matmuls) pile up.
            ghiT = cpool.tile([NH, G, P], F16)
            for pair in range(4):
                tp = pt.tile([NH, 2, P], F16, tag="tp")
                for qq in range(2):
                    g = pair * 2 + qq
                    nc.tensor.transpose(tp[:, qq, :], ghi[:, g, :], ident[:])
                nc.scalar.copy(ghiT[:, pair * 2:pair * 2 + 2, :], tp[:])
            yps = [None] * G
            for g in range(G):
                yp = py.tile([P, NLP], F32, tag="yp")
                nc.tensor.matmul(yp[:], lhsT=ghiT[:, g, :], rhs=a3_sb[:],
                                 start=True, stop=True)
                yps[g] = yp

            # glo[p,g,81c+M] = hv[p,g,c,8] * lo4[p,g,M]
            # groups 0-3 in one op on DVE; groups 4-7 per-group on GpSimd so
            # STT_g waits only on its own group's glo (a single Pool op made
            # STT4 stall ~0.5us on the whole second half)
            glo = cpool.tile([P, G, NL], F32)
            glo_out = glo[:].rearrange("p g (c M) -> p g c M", c=3)
            glo_hi = hv[:, :, :, 8][:, :, :, None].to_broadcast((P, G, 3, NH))
            glo_lo = lo4[:][:, :, None, :].to_broadcast((P, G, 3, NH))
            # Pool filler: the first Pool op after an idle gap pays ~0.9us of
            # restart cost; absorb it on a throwaway op (ready right after q)
            # so the glo ops below run at steady-state
            nc.gpsimd.tensor_mul(pwarm[:], q[:, 0, 0, 0:2], q[:, 0, 0, 0:2])
            for g in range(4, G):
                nc.gpsimd.tensor_mul(
                    glo[:, g, :].rearrange("p (c M) -> p c M", c=3),
                    hv[:, g, :, 8][:, :, None].to_broadcast((P, 3, NH)),
                    lo4[:, g, :][:, None, :].to_broadcast((P, 3, NH)))

            # out[:, g] = rowsum(Y_g * glo_g), fused; interleave the per-group
            # glo builds so STT0 starts as soon as glo_0 + Y_0 exist.  Ship
            # each result half as soon as it is done.
            res = cpool.tile([P, G], F32)
            wscr0 = cpool.tile([P, NL], F32)
            wscr1 = cpool.tile([P, NL], F32)
            out_pg = out_ext.rearrange("(p g) -> p g", g=G)

            nc.vector.tensor_mul(glo_out[:, 0:4], glo_hi[:, 0:4],
                                 glo_lo[:, 0:4])
            for g in range(G):
                wscr = wscr0 if g % 2 == 0 else wscr1
                nc.vector.scalar_tensor_tensor(
                    out=wscr[:], in0=glo[:, g, :], scalar=0.0,
                    in1=yps[g][:, 0:NL],
                    op0=mybir.AluOpType.add, op1=mybir.AluOpType.mult,
                    accum_out=res[:, g:g + 1])
                if g == 3:
                    nc.sync.dma_start(out_pg[:, 0:4], res[:, 0:4])
            nc.sync.dma_start(out_pg[:, 4:8], res[:, 4:8])

    nc.compile()
    return nc


def _get_program():
    global _PROGRAM
    if _PROGRAM is None:
        _PROGRAM = _build_program()
    return _PROGRAM


def kernel(adds, params, weights, params2):
    adds = np.ascontiguousarray(np.asarray(adds)[:, PERM], dtype=np.float32)
    A = _compute_A(params, weights, params2)
    nc = _get_program()
    in_maps = [
        {"adds": adds[i * B_LOC:(i + 1) * B_LOC], "amat": A}
        for i in range(N_CORES)
    ]
    results = bass_utils.run_bass_kernel_spmd(nc, in_maps, list(range(N_CORES))).results
    return np.concatenate([results[i]["out"] for i in range(N_CORES)])


# revision 34
# speedup vs baseline: 1.0264x; 1.0264x over previous
"""Trainium2 Bass kernel for nn_Model_22677427323544.

The circuit is AngleEmbedding(adds) followed by a batch-independent gate
sequence, then <Z_0>. Algebraically out[b] = r_b^T A r_b with A a fixed real
symmetric 512x512 matrix and r_b the real Kronecker vector of per-wire
(cos(t/2), sin(t/2)).

Each wire contributes a factor c^2, s^2, or c*s to every A[j,k] r_j r_k term,
so the quadratic form collapses to a LINEAR form over per-wire 3-vectors.
Since (c^2, s^2, c*s) = T (1, cos t, sin t) with a fixed 3x3 T, fold T into
the coefficient tensor on the host: with h_i[b] = (1, cos t_i, sin t_i),

    out[b] = < A3h , h_0[b] x h_1[b] x ... x h_8[b] >

Split wires 0-3 (81) / 4-8 (243):  out[b] = H_hi[b]^T A3h H_lo[b].
The h-basis needs NO device-side work beyond two Sin activations (the ones
plane is a memset), unlike the g-basis which needs squares/products.

Device (per core, 1024 samples = 128 partitions x 8 groups; wire slots
host-permuted to [0,5,2,7,1,6,3,8,4] so every kron level reads contiguous
slices):
  1. half-angle trig keeps Sin-table inputs in the proven range:
     w=sin(t/2), u=sin(t/4); cos t = 1-2w^2 (GpSimd), sin t = 2w(1-2u^2)
     (DVE).  A dummy activation hoists the 1.3us Sin ACT_TABLE_LOAD off the
     adds-DMA critical path.  hv[P,G,3,9] fp32, ones plane via memset.
  2. DVE krons (fp32 -- fp16 gives no DVE speedup on broadcast APs and
     ~20% penalty): q[P,G,4,9] wire-pairs (3 ops, ISA caps free dims at 3),
     ghi[P,G,81] (fp16 out, for the PE) + lo4[P,G,81] (fp32), glo =
     h4 x lo4 (groups 0-3 in one DVE op; 4-7 per-group on GpSimd, which
     runs ~570ns/group steady-state after a warm-up op absorbs the ~1.2us
     Q7 library-load penalty).
  3. all 8 PE transposes of ghi_g first (fp16 -> 1 cyc/row; emitting any
     copy-gated matmul earlier stalls the in-order PE) with per-2-group
     ScalarE PSUM->SBUF casts, then 8 fp16 matmuls Y_g = ghi_g^T @ A3h,
     one PSUM tile per group (a shared tile would make every STT wait on
     all 8 matmuls).
  4. per group: fused VectorE scalar_tensor_tensor dot rowsum(Y_g * glo_g);
     each half of res is DMAed out as soon as its STTs finish.
DMAs: adds dispatched on SP, A3h on ScalarE (parallel dispatch); out on SP.
The exec-time window is (last event incl ~9.9us of fixed out-DMA latency +
framework teardown) - (first engine instruction); the only controllable
term is input-DMA wait + the VectorE serial chain, which now runs gap-free.
"""
import math

import numpy as np

import concourse.bass as bass
import concourse.tile as tile
from concourse import bacc, mybir
from concourse import bass_utils

N_WIRES = 9
N_CORES = 8
B = 8192
B_LOC = B // N_CORES          # 1024
P = 128                       # partitions
G = B_LOC // P                # 8 batch groups per partition
NH = 81                       # 3^4, wires 0-3
NL = 243                      # 3^5, wires 4-8
NLP = 256                     # padded so each PSUM matmul slab is half a bank
F32 = mybir.dt.float32
F16 = mybir.dt.float16

# wire -> hv slot order: slots 0-3 = hi factors of pairs (0,1),(5,6),(2,3),
# (7,8); slots 4-7 = lo factors; slot 8 = wire 4.  This makes every kron
# level read contiguous slot slices (see _build_program).
PERM = [0, 5, 2, 7, 1, 6, 3, 8, 4]

# ---------------------------------------------------------------------------
# Host-side parameter folding: A = Re(D^H U^H Z0 U D), 3-ary fold, T fold
# ---------------------------------------------------------------------------

DIM = 1 << N_WIRES

_X = np.array([[0, 1], [1, 0]], dtype=np.complex128)
_CNOT = np.array(
    [[1, 0, 0, 0], [0, 1, 0, 0], [0, 0, 0, 1], [0, 0, 1, 0]], dtype=np.complex128
)


def _rx(t):
    c, s = np.cos(t / 2), np.sin(t / 2)
    return np.array([[c, -1j * s], [-1j * s, c]])


def _ry(t):
    c, s = np.cos(t / 2), np.sin(t / 2)
    return np.array([[c, -s], [s, c]], dtype=np.complex128)


def _rz(t):
    return np.array([[np.exp(-0.5j * t), 0], [0, np.exp(0.5j * t)]])


def _rot(phi, theta, omega):
    return _rz(omega) @ _ry(theta) @ _rz(phi)


def _crz(t):
    return np.diag([1, 1, np.exp(-0.5j * t), np.exp(0.5j * t)]).astype(np.complex128)


def _crx(t):
    m = np.eye(4, dtype=np.complex128)
    m[2:, 2:] = _rx(t)
    return m


def _apply_1q(state, U, w):
    s = np.moveaxis(state, 1 + w, -1)
    s = np.einsum('ij,...j->...i', U, s)
    return np.moveaxis(s, -1, 1 + w)


def _apply_2q(state, U, c, t):
    s = np.moveaxis(state, (1 + c, 1 + t), (-2, -1))
    shp = s.shape
    s = s.reshape(shp[:-2] + (4,))
    s = np.einsum('ij,...j->...i', U, s)
    return np.moveaxis(s.reshape(shp), (-2, -1), (1 + c, 1 + t))


def _entangle_block(state, p):
    j = 0
    for i in range(N_WIRES):
        ip = (i + 1) % N_WIRES
        state = _apply_1q(state, _ry(p[j]), i)
        state = _apply_1q(state, _ry(p[j + 1]), ip)
        state = _apply_2q(state, _CNOT, i, ip)
        state = _apply_2q(state, _crz(p[j + 2]), i, ip)
        state = _apply_1q(state, _X, ip)
        state = _apply_2q(state, _crx(p[j + 3]), i, ip)
        j += 4
    return state


def _sel_layer(state, w, r):
    for i in range(N_WIRES):
        state = _apply_1q(state, _rot(w[i, 0], w[i, 1], w[i, 2]), i)
    for i in range(N_WIRES):
        state = _apply_2q(state, _CNOT, i, (i + r) % N_WIRES)
    return state


def _compute_A(params, weights, params2):
    """Return the folded h-basis coefficient matrix A3h [81, 256] (fp16)."""
    params = np.asarray(params, np.float64)
    weights = np.asarray(weights, np.float64)
    params2 = np.asarray(params2, np.float64)
    state = np.eye(DIM, dtype=np.complex128).reshape((DIM,) + (2,) * N_WIRES)
    for l in range(3):
        state = _entangle_block(state, params[l * 36:(l + 1) * 36])
    for l in range(3):
        state = _sel_layer(state, weights[l], (l % (N_WIRES - 1)) + 1)
    for l in range(5):
        state = _entangle_block(state, params2[l * 36:(l + 1) * 36])
    U = state.reshape(DIM, DIM).T
    z = np.where(np.arange(DIM) < DIM // 2, 1.0, -1.0)
    M = U.conj().T @ (z[:, None] * U)
    pc = np.array([bin(j).count('1') for j in range(DIM)])
    d = (-1j) ** pc
    A = ((np.conj(d)[:, None] * M * d[None, :]).real).astype(np.float64)

    # fold 512x512 -> 3^9: digit 0 = (0,0), 1 = (1,1), 2 = (0,1)/(1,0)
    j = np.arange(DIM)
    jb = (j[:, None, None] >> (8 - np.arange(N_WIRES))[None, None, :]) & 1
    kb = (j[None, :, None] >> (8 - np.arange(N_WIRES))[None, None, :]) & 1
    digit = np.where((jb == 0) & (kb == 0), 0, np.where((jb == 1) & (kb == 1), 1, 2))
    m = np.zeros((DIM, DIM), np.int64)
    for i in range(N_WIRES):
        m = m * 3 + digit[:, :, i]
    A3 = np.zeros(3 ** N_WIRES)
    np.add.at(A3, m.ravel(), A.ravel())

    # change of basis per wire: g = (c^2, s^2, cs) = T (1, cos t, sin t)
    T = np.array([[.5, .5, 0.], [.5, -.5, 0.], [0., 0., .5]])
    A9 = A3.reshape((3,) * N_WIRES)
    for ax in range(N_WIRES):
        A9 = np.moveaxis(np.tensordot(A9, T, axes=([ax], [0])), -1, ax)
    A3h = A9.reshape(NH, NL)
    A3p = np.zeros((NH, NLP), np.float16)
    A3p[:, :NL] = A3h.astype(np.float16)
    return np.ascontiguousarray(A3p)


# ---------------------------------------------------------------------------
# Device program (per core: 1024 samples; sample index = p*G + g)
# ---------------------------------------------------------------------------

_PROGRAM = None


def _build_program():
    nc = bacc.Bacc("TRN2", target_bir_lowering=False, debug=False,
                   num_devices=N_CORES)
    adds_ext = nc.dram_tensor("adds", [B_LOC, N_WIRES], F32,
                              kind="ExternalInput").ap()
    amat_ext = nc.dram_tensor("amat", [NH, NLP], F16,
                              kind="ExternalInput").ap()
    out_ext = nc.dram_tensor("out", [B_LOC], F32, kind="ExternalOutput").ap()

    SIN = mybir.ActivationFunctionType.Sin

    with tile.TileContext(nc) as tc:
        with (
            tc.tile_pool(name="const", bufs=1) as cpool,
            tc.tile_pool(name="psum_t", bufs=2, space="PSUM") as pt,
            tc.tile_pool(name="psum_y", bufs=4, space="PSUM") as py,
        ):
            # input DMAs: adds on SP, A3h on Scalar -- parallel dispatch
            adds_sb = cpool.tile([P, G, N_WIRES], F32)
            nc.sync.dma_start(adds_sb[:], adds_ext.rearrange("(p g) i -> p g i", g=G))

            # identity for PE transpose (fp16 to match transposed data)
            ident = cpool.tile([P, P], F16)
            nc.gpsimd.memset(ident[:], 0.0)
            nc.gpsimd.affine_select(
                out=ident[:], in_=ident[:],
                compare_op=mybir.AluOpType.not_equal, fill=1.0,
                base=0, pattern=[[-1, P]], channel_multiplier=1)

            # hv[p,g,comp,slot]: comp 0 = 1, 1 = cos t, 2 = sin t
            hv = cpool.tile([P, G, 3, N_WIRES], F32)
            nc.gpsimd.memset(hv[:, :, 0, :], 1.0)

            # GpSimd warm-up: the first Pool tensor op pays a ~1.2us Q7
            # library-load penalty, and the first op of the glo shape pays
            # another ~0.9us first-use cost -- pay both here, during the DMA
            # wait, so the real glo ops run at steady-state ~570ns
            pwarm = cpool.tile([P, 2], F32)
            nc.gpsimd.tensor_mul(pwarm[:], hv[:, 0, 0, 0:2], hv[:, 0, 0, 0:2])
            a3_sb = cpool.tile([NH, NLP], F16)
            pwarm2 = cpool.tile([P, 3, NH], F32)
            nc.gpsimd.tensor_mul(
                pwarm2[:],
                hv[:, 0, 0, 0][:, None, None].to_broadcast((P, 3, NH)),
                hv[:, 0, 0, 1][:, None, None].to_broadcast((P, 3, NH)))
            nc.gpsimd.dma_start(a3_sb[:], amat_ext)

            # dummy activation on a ready tile: the act-table insert pass
            # attaches the following activation's waits to the table load, so
            # give it one with no data deps and the 1.3us Sin ACT_TABLE_LOAD
            # runs during the adds-DMA wait
            sdum = cpool.tile([P, 1], F32)
            nc.scalar.activation(sdum[:], ident[:, 0:1], SIN, scale=1.0)
            # a3 on GpSimd (emitted after the Pool warm-ups below): SP keeps
            # adds alone (a second SP dispatch interleaves descriptors and
            # delays the adds completion ~1.4us) and ScalarE keeps only the
            # table load + trig chain.  A3h is needed only by the matmuls at
            # ~13.5us, so Pool's slow ~0.9us dispatch is harmless.

            # half-angle trig (Sin inputs stay within the proven table range):
            # w = sin(t/2), u = sin(t/4); cos t = 1-2w^2,
            # sin t = 2w(1-2u^2).  u is computed FIRST so DVE's sin-t path
            # (usq, c2) overlaps the w activation; the cos-t path runs
            # entirely on ScalarE (Square, then Identity with scale/bias --
            # both live in the already-loaded trig table set).
            w = cpool.tile([P, G, N_WIRES], F32)
            u = cpool.tile([P, G, N_WIRES], F32)
            nc.scalar.activation(u[:], adds_sb[:], SIN, scale=0.25)
            nc.scalar.activation(w[:], adds_sb[:], SIN, scale=0.5)
            wsq = cpool.tile([P, G, N_WIRES], F32)
            usq = cpool.tile([P, G, N_WIRES], F32)
            c2 = cpool.tile([P, G, N_WIRES], F32)
            nc.scalar.square(wsq[:], w[:])
            nc.scalar.activation(hv[:, :, 1, :], wsq[:],
                                 mybir.ActivationFunctionType.Identity,
                                 scale=-2.0, bias=1.0)
            nc.vector.tensor_mul(usq[:], u[:], u[:])
            nc.vector.tensor_scalar(
                out=c2[:], in0=usq[:], scalar1=-2.0, scalar2=1.0,
                op0=mybir.AluOpType.mult, op1=mybir.AluOpType.add)
            nc.vector.scalar_tensor_tensor(
                out=hv[:, :, 2, :], in0=w[:], scalar=2.0, in1=c2[:],
                op0=mybir.AluOpType.mult, op1=mybir.AluOpType.mult)

            # q[p,g,j,3b+m] = hv[p,g,b,j] * hv[p,g,m,4+j], one DVE op per b
            # (DVE ISA caps free dims at 3).  j order: pairs (w0,w1),(w5,w6),
            # (w2,w3),(w7,w8).
            q = cpool.tile([P, G, 4, 9], F32)
            q_lo = hv[:, :, :, 4:8].rearrange("p g m j -> p g j m")
            q_b = q[:].rearrange("p g j (b m) -> p g j b m", b=3)
            # the b=0 plane is a multiply by ones, i.e. a copy: ScalarE
            # (idle here) does it while DVE does the two real products
            nc.scalar.copy(q_b[:, :, :, 0, :], q_lo)
            for b in (1, 2):
                q_hi = hv[:, :, b, 0:4][:, :, :, None].to_broadcast((P, G, 4, 3))
                nc.vector.tensor_mul(q_b[:, :, :, b, :], q_hi, q_lo)

            # ghi[p,g,9B+M] = q0[B]*q2[M] (digits d0d1 d2d3) -- fp16 so the
            # PE transposes run at 1 cyc/row instead of 2
            # lo4[p,g,9B+M] = q1[B]*q3[M] (digits d5d6 d7d8) -- fp32 for glo
            ghi = cpool.tile([P, G, NH], F16)
            lo4 = cpool.tile([P, G, NH], F32)
            for k, dst in ((0, ghi), (1, lo4)):
                rr_out = dst[:].rearrange("p g (B M) -> p g B M", B=9)
                rr_hi = q[:, :, k, :][:, :, :, None].to_broadcast((P, G, 9, 9))
                rr_lo = q[:, :, 2 + k, :][:, :, None, :].to_broadcast((P, G, 9, 9))
                nc.vector.tensor_mul(rr_out, rr_hi, rr_lo)

            # per 2 groups: PE transpose ghi_g (fp16, 1 cyc/row), cast to
            # fp16 SBUF on ScalarE.  All 8 transposes are emitted before any
            # matmul: they are cheap, and the in-order PE otherwise stalls on
            # a copy-gated matmul while later transposes (and hence later
            # copies and matmuls) pile up.
            ghiT = cpool.tile([NH, G, P], F16)
            for pair in range(4):
                tp = pt.tile([NH, 2, P], F16, tag="tp")
                for qq in range(2):
                    g = pair * 2 + qq
                    nc.tensor.transpose(tp[:, qq, :], ghi[:, g, :], ident[:])
                nc.scalar.copy(ghiT[:, pair * 2:pair * 2 + 2, :], tp[:])
            yps = [None] * G
            for g in range(G):
                yp = py.tile([P, NLP], F32, tag="yp")
                nc.tensor.matmul(yp[:], lhsT=ghiT[:, g, :], rhs=a3_sb[:],
                                 start=True, stop=True)
                yps[g] = yp

            # glo[p,g,81c+M] = hv[p,g,c,8] * lo4[p,g,M]
            # groups 0-3 in one op on DVE; groups 4-7 per-group on GpSimd so
            # STT_g waits only on its own group's glo (a single Pool op made
            # STT4 stall ~0.5us on the whole second half)
            glo = cpool.tile([P, G, NL], F32)
            glo_out = glo[:].rearrange("p g (c M) -> p g c M", c=3)
            glo_hi = hv[:, :, :, 8][:, :, :, None].to_broadcast((P, G, 3, NH))
            glo_lo = lo4[:][:, :, None, :].to_broadcast((P, G, 3, NH))
            # Pool filler: the first Pool op after an idle gap pays ~0.9us of
            # restart cost; absorb it on a throwaway op (ready right after q)
            # so the glo ops below run at steady-state
            nc.gpsimd.tensor_mul(pwarm[:], q[:, 0, 0, 0:2], q[:, 0, 0, 0:2])
            for g in range(4, G):
                nc.gpsimd.tensor_mul(
                    glo[:, g, :].rearrange("p (c M) -> p c M", c=3),
                    hv[:, g, :, 8][:, :, None].to_broadcast((P, 3, NH)),
                    lo4[:, g, :][:, None, :].to_broadcast((P, 3, NH)))

            # out[:, g] = rowsum(Y_g * glo_g), fused; interleave the per-group
            # glo builds so STT0 starts as soon as glo_0 + Y_0 exist.  Ship
            # each result half as soon as it is done.
            res = cpool.tile([P, G], F32)
            wscr0 = cpool.tile([P, NL], F32)
            wscr1 = cpool.tile([P, NL], F32)
            out_pg = out_ext.rearrange("(p g) -> p g", g=G)

            nc.vector.tensor_mul(glo_out[:, 0:4], glo_hi[:, 0:4],
                                 glo_lo[:, 0:4])
            for g in range(G):
                wscr = wscr0 if g % 2 == 0 else wscr1
                nc.vector.scalar_tensor_tensor(
                    out=wscr[:], in0=glo[:, g, :], scalar=0.0,
                    in1=yps[g][:, 0:NL],
                    op0=mybir.AluOpType.add, op1=mybir.AluOpType.mult,
                    accum_out=res[:, g:g + 1])
                if g == 3:
                    nc.sync.dma_start(out_pg[:, 0:4], res[:, 0:4])
            nc.sync.dma_start(out_pg[:, 4:8], res[:, 4:8])

    nc.compile()
    return nc


def _get_program():
    global _PROGRAM
    if _PROGRAM is None:
        _PROGRAM = _build_program()
    return _PROGRAM


def kernel(adds, params, weights, params2):
    adds = np.ascontiguousarray(np.asarray(adds)[:, PERM], dtype=np.float32)
    A = _compute_A(params, weights, params2)
    nc = _get_program()
    in_maps = [
        {"adds": adds[i * B_LOC:(i + 1) * B_LOC], "amat": A}
        for i in range(N_CORES)
    ]
    results = bass_utils.run_bass_kernel_spmd(nc, in_maps, list(range(N_CORES))).results
    return np.concatenate([results[i]["out"] for i in range(N_CORES)])


# revision 35
# speedup vs baseline: 1.0807x; 1.0530x over previous
"""Trainium2 Bass kernel for nn_Model_22677427323544.

The circuit is AngleEmbedding(adds) followed by a batch-independent gate
sequence, then <Z_0>. Algebraically out[b] = r_b^T A r_b with A a fixed real
symmetric 512x512 matrix and r_b the real Kronecker vector of per-wire
(cos(t/2), sin(t/2)).

Each wire contributes a factor c^2, s^2, or c*s to every A[j,k] r_j r_k term,
so the quadratic form collapses to a LINEAR form over per-wire 3-vectors.
Since (c^2, s^2, c*s) = T (1, cos t, sin t) with a fixed 3x3 T, fold T into
the coefficient tensor on the host: with h_i[b] = (1, cos t_i, sin t_i),

    out[b] = < A3h , h_0[b] x h_1[b] x ... x h_8[b] >

Split wires 0-3 (81) / 4-8 (243):  out[b] = H_hi[b]^T A3h H_lo[b].
The h-basis needs NO device-side work beyond two Sin activations (the ones
plane is a memset), unlike the g-basis which needs squares/products.

Device (per core, 1024 samples = 128 partitions x 8 groups; wire slots
host-permuted to [0,5,2,7,1,6,3,8,4] so every kron level reads contiguous
slices):
  1. half-angle trig keeps Sin-table inputs in the proven range:
     w=sin(t/2), u=sin(t/4); cos t = 1-2w^2 (GpSimd), sin t = 2w(1-2u^2)
     (DVE).  A dummy activation hoists the 1.3us Sin ACT_TABLE_LOAD off the
     adds-DMA critical path.  hv[P,G,3,9] fp32, ones plane via memset.
  2. DVE krons (fp32 -- fp16 gives no DVE speedup on broadcast APs and
     ~20% penalty): q[P,G,4,9] wire-pairs (3 ops, ISA caps free dims at 3),
     ghi[P,G,81] (fp16 out, for the PE) + lo4[P,G,81] (fp32), glo =
     h4 x lo4 (groups 0-3 in one DVE op; 4-7 per-group on GpSimd, which
     runs ~570ns/group steady-state after a warm-up op absorbs the ~1.2us
     Q7 library-load penalty).
  3. all 8 PE transposes of ghi_g first (fp16 -> 1 cyc/row; emitting any
     copy-gated matmul earlier stalls the in-order PE) with per-2-group
     ScalarE PSUM->SBUF casts, then 8 fp16 matmuls Y_g = ghi_g^T @ A3h,
     one PSUM tile per group (a shared tile would make every STT wait on
     all 8 matmuls).
  4. per group: fused VectorE scalar_tensor_tensor dot rowsum(Y_g * glo_g);
     each half of res is DMAed out as soon as its STTs finish.
DMAs: adds dispatched on SP, A3h on ScalarE (parallel dispatch); out on SP.
The exec-time window is (last event incl ~9.9us of fixed out-DMA latency +
framework teardown) - (first engine instruction); the only controllable
term is input-DMA wait + the VectorE serial chain, which now runs gap-free.
"""
import math

import numpy as np

import concourse.bass as bass
import concourse.tile as tile
from concourse import bacc, mybir
from concourse import bass_utils

N_WIRES = 9
N_CORES = 8
B = 8192
B_LOC = B // N_CORES          # 1024
P = 128                       # partitions
G = B_LOC // P                # 8 batch groups per partition
NH = 81                       # 3^4, wires 0-3
NL = 243                      # 3^5, wires 4-8
NLP = 256                     # padded so each PSUM matmul slab is half a bank
F32 = mybir.dt.float32
F16 = mybir.dt.float16

# wire -> hv slot order: slots 0-3 = hi factors of pairs (0,1),(5,6),(2,3),
# (7,8); slots 4-7 = lo factors; slot 8 = wire 4.  This makes every kron
# level read contiguous slot slices (see _build_program).
PERM = [0, 5, 2, 7, 1, 6, 3, 8, 4]

# ---------------------------------------------------------------------------
# Host-side parameter folding: A = Re(D^H U^H Z0 U D), 3-ary fold, T fold
# ---------------------------------------------------------------------------

DIM = 1 << N_WIRES

_X = np.array([[0, 1], [1, 0]], dtype=np.complex128)
_CNOT = np.array(
    [[1, 0, 0, 0], [0, 1, 0, 0], [0, 0, 0, 1], [0, 0, 1, 0]], dtype=np.complex128
)


def _rx(t):
    c, s = np.cos(t / 2), np.sin(t / 2)
    return np.array([[c, -1j * s], [-1j * s, c]])


def _ry(t):
    c, s = np.cos(t / 2), np.sin(t / 2)
    return np.array([[c, -s], [s, c]], dtype=np.complex128)


def _rz(t):
    return np.array([[np.exp(-0.5j * t), 0], [0, np.exp(0.5j * t)]])


def _rot(phi, theta, omega):
    return _rz(omega) @ _ry(theta) @ _rz(phi)


def _crz(t):
    return np.diag([1, 1, np.exp(-0.5j * t), np.exp(0.5j * t)]).astype(np.complex128)


def _crx(t):
    m = np.eye(4, dtype=np.complex128)
    m[2:, 2:] = _rx(t)
    return m


def _apply_1q(state, U, w):
    s = np.moveaxis(state, 1 + w, -1)
    s = np.einsum('ij,...j->...i', U, s)
    return np.moveaxis(s, -1, 1 + w)


def _apply_2q(state, U, c, t):
    s = np.moveaxis(state, (1 + c, 1 + t), (-2, -1))
    shp = s.shape
    s = s.reshape(shp[:-2] + (4,))
    s = np.einsum('ij,...j->...i', U, s)
    return np.moveaxis(s.reshape(shp), (-2, -1), (1 + c, 1 + t))


def _entangle_block(state, p):
    j = 0
    for i in range(N_WIRES):
        ip = (i + 1) % N_WIRES
        state = _apply_1q(state, _ry(p[j]), i)
        state = _apply_1q(state, _ry(p[j + 1]), ip)
        state = _apply_2q(state, _CNOT, i, ip)
        state = _apply_2q(state, _crz(p[j + 2]), i, ip)
        state = _apply_1q(state, _X, ip)
        state = _apply_2q(state, _crx(p[j + 3]), i, ip)
        j += 4
    return state


def _sel_layer(state, w, r):
    for i in range(N_WIRES):
        state = _apply_1q(state, _rot(w[i, 0], w[i, 1], w[i, 2]), i)
    for i in range(N_WIRES):
        state = _apply_2q(state, _CNOT, i, (i + r) % N_WIRES)
    return state


def _compute_A(params, weights, params2):
    """Return the folded h-basis coefficient matrix A3h [81, 256] (fp16)."""
    params = np.asarray(params, np.float64)
    weights = np.asarray(weights, np.float64)
    params2 = np.asarray(params2, np.float64)
    state = np.eye(DIM, dtype=np.complex128).reshape((DIM,) + (2,) * N_WIRES)
    for l in range(3):
        state = _entangle_block(state, params[l * 36:(l + 1) * 36])
    for l in range(3):
        state = _sel_layer(state, weights[l], (l % (N_WIRES - 1)) + 1)
    for l in range(5):
        state = _entangle_block(state, params2[l * 36:(l + 1) * 36])
    U = state.reshape(DIM, DIM).T
    z = np.where(np.arange(DIM) < DIM // 2, 1.0, -1.0)
    M = U.conj().T @ (z[:, None] * U)
    pc = np.array([bin(j).count('1') for j in range(DIM)])
    d = (-1j) ** pc
    A = ((np.conj(d)[:, None] * M * d[None, :]).real).astype(np.float64)

    # fold 512x512 -> 3^9: digit 0 = (0,0), 1 = (1,1), 2 = (0,1)/(1,0)
    j = np.arange(DIM)
    jb = (j[:, None, None] >> (8 - np.arange(N_WIRES))[None, None, :]) & 1
    kb = (j[None, :, None] >> (8 - np.arange(N_WIRES))[None, None, :]) & 1
    digit = np.where((jb == 0) & (kb == 0), 0, np.where((jb == 1) & (kb == 1), 1, 2))
    m = np.zeros((DIM, DIM), np.int64)
    for i in range(N_WIRES):
        m = m * 3 + digit[:, :, i]
    A3 = np.zeros(3 ** N_WIRES)
    np.add.at(A3, m.ravel(), A.ravel())

    # change of basis per wire: g = (c^2, s^2, cs) = T (1, cos t, sin t)
    T = np.array([[.5, .5, 0.], [.5, -.5, 0.], [0., 0., .5]])
    A9 = A3.reshape((3,) * N_WIRES)
    for ax in range(N_WIRES):
        A9 = np.moveaxis(np.tensordot(A9, T, axes=([ax], [0])), -1, ax)
    A3h = A9.reshape(NH, NL)
    A3p = np.zeros((NH, NLP), np.float16)
    A3p[:, :NL] = A3h.astype(np.float16)
    return np.ascontiguousarray(A3p)


# ---------------------------------------------------------------------------
# Device program (per core: 1024 samples; sample index = p*G + g)
# ---------------------------------------------------------------------------

_PROGRAM = None


def _build_program():
    nc = bacc.Bacc("TRN2", target_bir_lowering=False, debug=False,
                   num_devices=N_CORES)
    adds_ext = nc.dram_tensor("adds", [B_LOC, N_WIRES], F32,
                              kind="ExternalInput").ap()
    amat_ext = nc.dram_tensor("amat", [NH, NLP], F16,
                              kind="ExternalInput").ap()
    out_ext = nc.dram_tensor("out", [B_LOC], F32, kind="ExternalOutput").ap()

    SIN = mybir.ActivationFunctionType.Sin

    with tile.TileContext(nc) as tc:
        with (
            tc.tile_pool(name="const", bufs=1) as cpool,
            tc.tile_pool(name="psum_t", bufs=2, space="PSUM") as pt,
            tc.tile_pool(name="psum_y", bufs=4, space="PSUM") as py,
        ):
            # input DMAs: adds on SP, A3h on Scalar -- parallel dispatch
            adds_sb = cpool.tile([P, G, N_WIRES], F32)
            nc.sync.dma_start(adds_sb[:], adds_ext.rearrange("(p g) i -> p g i", g=G))

            # identity for PE transpose (fp16 to match transposed data)
            ident = cpool.tile([P, P], F16)
            nc.gpsimd.memset(ident[:], 0.0)
            nc.gpsimd.affine_select(
                out=ident[:], in_=ident[:],
                compare_op=mybir.AluOpType.not_equal, fill=1.0,
                base=0, pattern=[[-1, P]], channel_multiplier=1)

            # hv[p,g,comp,slot]: comp 0 = 1, 1 = cos t, 2 = sin t
            hv = cpool.tile([P, G, 3, N_WIRES], F32)
            nc.gpsimd.memset(hv[:, :, 0, :], 1.0)

            # GpSimd warm-up: the first Pool tensor op pays a ~1.2us Q7
            # library-load penalty, and the first op of the glo shape pays
            # another ~0.9us first-use cost -- pay both here, during the DMA
            # wait, so the real glo ops run at steady-state ~570ns
            pwarm = cpool.tile([P, 2], F32)
            nc.gpsimd.tensor_mul(pwarm[:], hv[:, 0, 0, 0:2], hv[:, 0, 0, 0:2])
            a3_sb = cpool.tile([NH, NLP], F16)
            pwarm2 = cpool.tile([P, 3, NH], F32)
            nc.gpsimd.tensor_mul(
                pwarm2[:],
                hv[:, 0, 0, 0][:, None, None].to_broadcast((P, 3, NH)),
                hv[:, 0, 0, 1][:, None, None].to_broadcast((P, 3, NH)))
            nc.gpsimd.dma_start(a3_sb[:], amat_ext)

            # dummy activation on a ready tile: the act-table insert pass
            # attaches the following activation's waits to the table load, so
            # give it one with no data deps and the 1.3us Sin ACT_TABLE_LOAD
            # runs during the adds-DMA wait
            sdum = cpool.tile([P, 1], F32)
            nc.scalar.activation(sdum[:], ident[:, 0:1], SIN, scale=1.0)
            # a3 on GpSimd (emitted after the Pool warm-ups below): SP keeps
            # adds alone (a second SP dispatch interleaves descriptors and
            # delays the adds completion ~1.4us) and ScalarE keeps only the
            # table load + trig chain.  A3h is needed only by the matmuls at
            # ~13.5us, so Pool's slow ~0.9us dispatch is harmless.

            # half-angle trig (Sin inputs stay within the proven table range):
            # w = sin(t/2), u = sin(t/4); cos t = 1-2w^2,
            # sin t = 2w(1-2u^2).  u is computed FIRST so DVE's sin-t path
            # (usq, c2) overlaps the w activation; the cos-t path runs
            # entirely on ScalarE (Square, then Identity with scale/bias --
            # both live in the already-loaded trig table set).
            w = cpool.tile([P, G, N_WIRES], F32)
            u = cpool.tile([P, G, N_WIRES], F32)
            nc.scalar.activation(u[:], adds_sb[:], SIN, scale=0.25)
            nc.scalar.activation(w[:], adds_sb[:], SIN, scale=0.5)
            wsq = cpool.tile([P, G, N_WIRES], F32)
            usq = cpool.tile([P, G, N_WIRES], F32)
            c2 = cpool.tile([P, G, N_WIRES], F32)
            # cos-t path on GpSimd in parallel with DVE's sin-t path
            nc.gpsimd.tensor_mul(wsq[:], w[:], w[:])
            nc.gpsimd.tensor_scalar(
                out=hv[:, :, 1, :], in0=wsq[:], scalar1=-2.0, scalar2=1.0,
                op0=mybir.AluOpType.mult, op1=mybir.AluOpType.add)
            nc.vector.tensor_mul(usq[:], u[:], u[:])
            nc.vector.tensor_scalar(
                out=c2[:], in0=usq[:], scalar1=-2.0, scalar2=1.0,
                op0=mybir.AluOpType.mult, op1=mybir.AluOpType.add)
            nc.vector.scalar_tensor_tensor(
                out=hv[:, :, 2, :], in0=w[:], scalar=2.0, in1=c2[:],
                op0=mybir.AluOpType.mult, op1=mybir.AluOpType.mult)

            # q[p,g,j,3b+m] = hv[p,g,b,j] * hv[p,g,m,4+j], one DVE op per b
            # (DVE ISA caps free dims at 3).  j order: pairs (w0,w1),(w5,w6),
            # (w2,w3),(w7,w8).
            q = cpool.tile([P, G, 4, 9], F32)
            q_lo = hv[:, :, :, 4:8].rearrange("p g m j -> p g j m")
            q_b = q[:].rearrange("p g j (b m) -> p g j b m", b=3)
            # the b=0 plane is a multiply by ones, i.e. a copy: ScalarE
            # (idle here) does it while DVE does the two real products
            nc.scalar.copy(q_b[:, :, :, 0, :], q_lo)
            for b in (1, 2):
                q_hi = hv[:, :, b, 0:4][:, :, :, None].to_broadcast((P, G, 4, 3))
                nc.vector.tensor_mul(q_b[:, :, :, b, :], q_hi, q_lo)

            # ghi[p,g,9B+M] = q0[B]*q2[M] (digits d0d1 d2d3) -- fp16 so the
            # PE transposes run at 1 cyc/row instead of 2
            # lo4[p,g,9B+M] = q1[B]*q3[M] (digits d5d6 d7d8) -- fp32 for glo
            ghi = cpool.tile([P, G, NH], F16)
            lo4 = cpool.tile([P, G, NH], F32)
            for k, dst in ((0, ghi), (1, lo4)):
                rr_out = dst[:].rearrange("p g (B M) -> p g B M", B=9)
                rr_hi = q[:, :, k, :][:, :, :, None].to_broadcast((P, G, 9, 9))
                rr_lo = q[:, :, 2 + k, :][:, :, None, :].to_broadcast((P, G, 9, 9))
                nc.vector.tensor_mul(rr_out, rr_hi, rr_lo)

            # per 2 groups: PE transpose ghi_g (fp16, 1 cyc/row), cast to
            # fp16 SBUF on ScalarE.  All 8 transposes are emitted before any
            # matmul: they are cheap, and the in-order PE otherwise stalls on
            # a copy-gated matmul while later transposes (and hence later
            # copies and matmuls) pile up.
            ghiT = cpool.tile([NH, G, P], F16)
            for pair in range(4):
                tp = pt.tile([NH, 2, P], F16, tag="tp")
                for qq in range(2):
                    g = pair * 2 + qq
                    nc.tensor.transpose(tp[:, qq, :], ghi[:, g, :], ident[:])
                nc.scalar.copy(ghiT[:, pair * 2:pair * 2 + 2, :], tp[:])
            yps = [None] * G
            for g in range(G):
                yp = py.tile([P, NLP], F32, tag="yp")
                nc.tensor.matmul(yp[:], lhsT=ghiT[:, g, :], rhs=a3_sb[:],
                                 start=True, stop=True)
                yps[g] = yp

            # glo[p,g,81c+M] = hv[p,g,c,8] * lo4[p,g,M]
            # groups 0-3 in one op on DVE; groups 4-7 per-group on GpSimd so
            # STT_g waits only on its own group's glo (a single Pool op made
            # STT4 stall ~0.5us on the whole second half)
            glo = cpool.tile([P, G, NL], F32)
            glo_out = glo[:].rearrange("p g (c M) -> p g c M", c=3)
            glo_hi = hv[:, :, :, 8][:, :, :, None].to_broadcast((P, G, 3, NH))
            glo_lo = lo4[:][:, :, None, :].to_broadcast((P, G, 3, NH))
            # Pool filler: the first Pool op after an idle gap pays ~0.9us of
            # restart cost; absorb it on a throwaway op (ready right after q)
            # so the glo ops below run at steady-state
            nc.gpsimd.tensor_mul(pwarm[:], q[:, 0, 0, 0:2], q[:, 0, 0, 0:2])
            for g in range(4, G):
                nc.gpsimd.tensor_mul(
                    glo[:, g, :].rearrange("p (c M) -> p c M", c=3),
                    hv[:, g, :, 8][:, :, None].to_broadcast((P, 3, NH)),
                    lo4[:, g, :][:, None, :].to_broadcast((P, 3, NH)))

            # out[:, g] = rowsum(Y_g * glo_g), fused; interleave the per-group
            # glo builds so STT0 starts as soon as glo_0 + Y_0 exist.  Ship
            # each result half as soon as it is done.
            res = cpool.tile([P, G], F32)
            wscr0 = cpool.tile([P, NL], F32)
            wscr1 = cpool.tile([P, NL], F32)
            out_pg = out_ext.rearrange("(p g) -> p g", g=G)

            nc.vector.tensor_mul(glo_out[:, 0:4], glo_hi[:, 0:4],
                                 glo_lo[:, 0:4])
            for g in range(G):
                wscr = wscr0 if g % 2 == 0 else wscr1
                nc.vector.scalar_tensor_tensor(
                    out=wscr[:], in0=glo[:, g, :], scalar=0.0,
                    in1=yps[g][:, 0:NL],
                    op0=mybir.AluOpType.add, op1=mybir.AluOpType.mult,
                    accum_out=res[:, g:g + 1])
                if g == 3:
                    nc.sync.dma_start(out_pg[:, 0:4], res[:, 0:4])
            nc.sync.dma_start(out_pg[:, 4:8], res[:, 4:8])

    nc.compile()
    return nc


def _get_program():
    global _PROGRAM
    if _PROGRAM is None:
        _PROGRAM = _build_program()
    return _PROGRAM


def kernel(adds, params, weights, params2):
    adds = np.ascontiguousarray(np.asarray(adds)[:, PERM], dtype=np.float32)
    A = _compute_A(params, weights, params2)
    nc = _get_program()
    in_maps = [
        {"adds": adds[i * B_LOC:(i + 1) * B_LOC], "amat": A}
        for i in range(N_CORES)
    ]
    results = bass_utils.run_bass_kernel_spmd(nc, in_maps, list(range(N_CORES))).results
    return np.concatenate([results[i]["out"] for i in range(N_CORES)])
